# revision 23
# baseline (speedup 1.0000x reference)
"""CPPN MLP (12 -> 32 -> 32 -> 32 -> 3, per-node activations) on 8 TRN2 cores.

Data-parallel over the pixel axis; fp16 hidden state / weights (PE full-rate,
N=1024 moving), fp32 PSUM accumulation.  Each core processes P_CORE pixels as
4 pixel-groups packed on SBUF partitions; hidden tile layout per layer is
[4 groups x 32 nodes] with nodes class-sorted [sin | gauss | rest] so every
per-class operation is a prefix / in-place range:

  rest (tanh/sigmoid/identity): one Tanh pass over all 128 rows with
      per-partition scale/bias operand columns:
        sigmoid(z) = 0.5*tanh(z/2) + 0.5          (affine folded downstream)
        identity(z) = tanh(eps*z)/eps             (1/eps folded downstream)
  gauss: in-place Square pass on the gauss rows of the PSUM tile
      (u = (z+b)^2/2 via the Square affine), then the SAME main Tanh pass:
        exp(-u) ~= GB + GA*tanh(GC*u + GD)        (minimax fit, |err|<6e-3;
      GA/GB folded downstream)
  sin: DVE range reduction (3 ops: magic-constant round-to-nearest of
      (z+b)/2pi, then a one-term Cody-Waite ur = z - 2pi*k), then a Sin pass
      stored = sin(ur + b) = sin(z+b)  (b via the Sin bias operand).

The sin + gauss work is PACKED across chunk pairs: a small aux matmul (a
column slice of the main stationary) re-emits the sin+gauss pre-activations
of two consecutive chunks into one [128, chunk] PSUM tile (64-row slots).
The DVE range reduction, the in-place Square and the Sin pass then run once
per pair instead of once per chunk; squared gauss values DMA-scatter back
into each chunk's ps tile (PSUM dst: no partition-alignment constraint) to
ride the main Tanh pass, and sin results DMA-scatter into each chunk's
h[0:a) prefix.
"""

import os
import sys

import numpy as np

_REPO = "/root/.axon_site/_ro/trn_rl_repo"
if _REPO not in sys.path and not os.path.isdir("/opt/trn_rl_repo"):
    sys.path.insert(0, _REPO)

import concourse.bacc as bacc
import concourse.bass as bass  # noqa: F401
import concourse.tile as tile
from concourse import mybir
from concourse.bass_utils import run_bass_kernel_spmd

# Pin the activation-function table to the single set containing every
# function this kernel uses ({Tanh, Square, Sin}).  Without this, bacc's
# greedy per-instruction set selection can alternate sets and emit an
# ACT_TABLE_LOAD (~2.7us) per chunk.
_orig_get_tables = bacc.get_activation_tables


def _pinned_tables(arch):
    t = _orig_get_tables(arch)
    if "silu_and_others" in t:
        # act_func_set_id is the POSITION in act_info.json's set list, so
        # keep every entry (order intact) and just empty the others.
        return {name: (funcs if name == "silu_and_others" else set())
                for name, funcs in t.items()}
    return t


bacc.get_activation_tables = _pinned_tables

F32 = mybir.dt.float32
F16 = mybir.dt.float16

P_TOTAL = 1024 * 1024
N_IN, H, N_OUT = 12, 32, 3
N_CORES = 8
P_CORE = P_TOTAL // N_CORES  # 131072
G = 4                        # pixel groups packed on partitions
PG = P_CORE // G             # 32768 pixels per group per core
CHUNK = 1024                 # pixels per group per chunk

TWO_PI = float(np.float32(2.0 * np.pi))
INV_2PI = float(np.float32(1.0 / (2.0 * np.pi)))
MAGIC = float(np.float32(1.5 * 2.0 ** 23))
ID_EPS = 2.0 ** -10          # identity-via-tanh input scale
# exp(-u) ~= GB + GA*tanh(GC*u + GD) minimax fit on u in [0, inf)
GA, GB, GC, GD = -3.17748316, 3.18342043, 0.55705346, 0.84609093
SQRT_HALF = float(np.sqrt(0.5))


# class codes: 0 = sin, 1 = gauss, 2 = rest (identity/tanh/sigmoid)
def _cls_of_act(a):
    return {3: 0, 4: 1}.get(int(a), 2)


def _sorted_layout(act):
    """Order the H nodes by [sin | gauss | rest]; return (perm, n_sin,
    n_gauss).  perm[j] = original node index placed at sorted slot j."""
    cls = np.array([_cls_of_act(a) for a in act])
    perm = np.argsort(cls, kind="stable")
    return perm, int((cls == 0).sum()), int((cls == 1).sum())


class _Plan:
    """Host-side folded weights + per-layer layouts. All float64 math."""

    def __init__(self, bias_in, W1, b1, act1, W2, b2, act2, W3, b3, act3,
                 Wout, bout):
        layers = [(W1, b1, act1), (W2, b2, act2), (W3, b3, act3)]
        self.perms, self.nsin, self.ngauss = [], [], []
        self.lhsT = []          # device stationary matrices (np.float16)
        self.cols = []          # per-layer dict of [128] operand columns
        # incoming per-node output transform: h_true = alpha*stored + beta
        in_alpha = np.ones(N_IN, dtype=np.float64)
        in_beta = np.asarray(bias_in, dtype=np.float64)  # h0 = x + bias_in
        in_dim = N_IN
        in_layout = None  # for L1 the input layout is the fixed feature order

        for li, (W, b, act) in enumerate(layers):
            W = np.asarray(W, dtype=np.float64)
            b = np.asarray(b, dtype=np.float64)
            act = np.asarray(act)
            perm, ns, ng = _sorted_layout(act)
            self.perms.append(perm)
            self.nsin.append(ns)
            self.ngauss.append(ng)

            # effective weights / bias absorbing incoming transforms
            W_eff = W * in_alpha[:, None]                  # [in_dim, H]
            b_eff = b + in_beta @ W                        # [H]

            # device stationary: block diagonal over groups with node sort
            K = G * in_dim
            lt = np.zeros((K, 128), dtype=np.float64)
            for g in range(G):
                for j in range(H):
                    node = perm[j]
                    m = self._row(g, j)
                    if li == 0:
                        rows = np.arange(in_dim) + in_dim * g
                        lt[rows, m] = W_eff[:, node]
                    else:
                        for k_in in range(in_dim):
                            kpart = in_layout[g][k_in]
                            lt[kpart, m] = W_eff[k_in, node]
            self.lhsT.append(lt.astype(np.float16))

            # activation operand columns, indexed by device partition
            tanh_scale = np.zeros(128, dtype=np.float64)
            tanh_bias = np.zeros(128, dtype=np.float64)
            sq_scale = np.zeros(128, dtype=np.float64)
            sq_bias = np.zeros(128, dtype=np.float64)
            sin_c = np.zeros(128, dtype=np.float64)
            sin_bias = np.zeros(128, dtype=np.float64)
            out_alpha = np.ones(H, dtype=np.float64)
            out_beta = np.zeros(H, dtype=np.float64)
            for j in range(H):
                node = perm[j]
                a = int(act[node])
                be = b_eff[node]
                for g in range(G):
                    m = self._row(g, j)
                    if a == 1:        # tanh
                        tanh_scale[m] = 1.0
                        tanh_bias[m] = be
                    elif a == 2:      # sigmoid -> tanh(z/2)
                        tanh_scale[m] = 0.5
                        tanh_bias[m] = 0.5 * be
                    elif a == 0:      # identity -> tanh(eps*z)
                        tanh_scale[m] = float(ID_EPS)
                        tanh_bias[m] = float(ID_EPS) * be
                    elif a == 3:      # sin: magic round + Cody-Waite on DVE
                        sin_c[m] = be
                        sin_bias[m] = be
                    elif a == 4:      # gauss: u = ((z+b)/sqrt2)^2, then
                        sq_scale[m] = SQRT_HALF          # tanh(GC*u + GD)
                        sq_bias[m] = SQRT_HALF * be
                        tanh_scale[m] = GC
                        tanh_bias[m] = GD
                if a == 1:
                    out_alpha[node], out_beta[node] = 1.0, 0.0
                elif a == 2:
                    out_alpha[node], out_beta[node] = 0.5, 0.5
                elif a == 0:
                    out_alpha[node], out_beta[node] = 1.0 / float(ID_EPS), 0.0
                elif a == 3:
                    out_alpha[node], out_beta[node] = 1.0, 0.0
                elif a == 4:
                    out_alpha[node], out_beta[node] = GA, GB
            # dense packed-tile operand columns: the sin tile packs slots
            # of 4ns rows back to back; the gauss tile slots of 4ng rows.
            s_sinc = np.zeros(128, dtype=np.float64)
            s_sinb = np.zeros(128, dtype=np.float64)
            if ns:
                for k in range(128 // (4 * ns)):
                    for r in range(4 * ns):
                        s_sinc[4 * ns * k + r] = sin_c[r]
                        s_sinb[4 * ns * k + r] = sin_bias[r]
            g_sqs = np.zeros(128, dtype=np.float64)
            g_sqb = np.zeros(128, dtype=np.float64)
            if ng:
                for k in range(128 // (4 * ng)):
                    for r in range(4 * ng):
                        g_sqs[4 * ng * k + r] = sq_scale[4 * ns + r]
                        g_sqb[4 * ng * k + r] = sq_bias[4 * ns + r]
            self.cols.append({
                "tanh_scale": tanh_scale, "tanh_bias": tanh_bias,
                "s_sinc": s_sinc, "s_sinb": s_sinb,
                "g_sqs": g_sqs, "g_sqb": g_sqb,
            })

            # next layer's incoming transform, in SORTED node order, and the
            # partition layout of this layer's output for the next lhsT build.
            in_dim = H
            in_layout = [[self._row(g, j) for j in range(H)]
                         for g in range(G)]
            in_alpha = out_alpha[perm]
            in_beta = out_beta[perm]
            if li < 2:
                layers[li + 1] = (np.asarray(layers[li + 1][0])[perm, :],
                                  layers[li + 1][1], layers[li + 1][2])
            else:
                self._wout_perm = perm

        # output layer
        Wo = np.asarray(Wout, dtype=np.float64)[self._wout_perm, :]
        bo = np.asarray(bout, dtype=np.float64)
        Wo_eff = Wo * in_alpha[:, None]
        bo_eff = bo + in_beta @ Wo
        lt = np.zeros((128, 32), dtype=np.float64)
        for g in range(G):
            for j in range(H):
                kpart = in_layout[g][j]
                for o in range(N_OUT):
                    lt[kpart, 3 * g + o] = Wo_eff[j, o]
        self.lhsT_out = lt.astype(np.float16)
        out_bias = np.zeros(128, dtype=np.float64)
        for q in range(4):
            for g in range(G):
                for o in range(N_OUT):
                    out_bias[32 * q + 3 * g + o] = bo_eff[o]
        self.out_bias = out_bias

        # pack all operand columns into one [128, 16] fp32 block
        colblk = np.zeros((128, 32), dtype=np.float64)
        for li in range(3):
            c = self.cols[li]
            colblk[:, 8 * li + 0] = c["tanh_scale"]
            colblk[:, 8 * li + 1] = c["tanh_bias"]
            colblk[:, 8 * li + 2] = c["s_sinc"]
            colblk[:, 8 * li + 3] = c["s_sinb"]
            colblk[:, 8 * li + 4] = c["g_sqs"]
            colblk[:, 8 * li + 5] = c["g_sqb"]
        colblk[:, 24] = self.out_bias
        colblk[:, 25] = GC
        colblk[:, 26] = GD
        self.colblk = colblk.astype(np.float32)

    @staticmethod
    def _row(g, j):
        """Device partition of sorted-slot j, group g.  Rows are class-sorted
        ACROSS groups: slot j occupies partitions 4*j + g."""
        return 4 * j + g


def _build_program(nsin, ngauss, p_core=P_CORE, chunk=CHUNK):
    """Build the bass module.  Program structure depends only on the
    per-layer (n_sin, n_gauss) counts, not on weight values."""
    pg = p_core // G
    nchunk = pg // chunk
    assert pg % chunk == 0 and nchunk % 8 == 0

    nc = bacc.Bacc("TRN2", target_bir_lowering=False, debug=False,
                   num_devices=N_CORES)
    xT = nc.dram_tensor("xT", [G * N_IN, pg], F16, kind="ExternalInput").ap()
    wst = nc.dram_tensor("wst", [128, 416], F16, kind="ExternalInput").ap()
    cst = nc.dram_tensor("cst", [128, 32], F32, kind="ExternalInput").ap()
    yT = nc.dram_tensor("yT", [128, pg // 4], F32, kind="ExternalOutput").ap()

    with tile.TileContext(nc) as tc:
        cpool = tc.alloc_tile_pool(name="consts", bufs=1)
        wst_t = cpool.tile([128, 416], F16, tag="wst")
        cc_t = cpool.tile([128, 32], F32, tag="cc")
        nc.sync.dma_start(out=wst_t[:], in_=wst[:])
        nc.sync.dma_start(out=cc_t[:], in_=cst[:])
        w_tiles = [wst_t[:, 0:128], wst_t[:, 128:256], wst_t[:, 256:384]]
        wo_t = wst_t[:, 384:416]
        col_t = cc_t[:, 0:32]

        xpool = tc.alloc_tile_pool(name="xin", bufs=3)
        hpool = tc.alloc_tile_pool(name="h", bufs=28)
        spool = tc.alloc_tile_pool(name="scratch", bufs=5)
        zbpool = tc.alloc_tile_pool(name="zb", bufs=3)
        sgpool = tc.alloc_tile_pool(name="sing", bufs=3)
        srpool = tc.alloc_tile_pool(name="sinr", bufs=3)
        gppool = tc.alloc_tile_pool(name="gaup", bufs=3)
        grpool = tc.alloc_tile_pool(name="gaur", bufs=3)
        opool = tc.alloc_tile_pool(name="osb", bufs=2)
        ppool = tc.alloc_tile_pool(name="psum", bufs=3, space="PSUM")
        oppool = tc.alloc_tile_pool(name="psum_o", bufs=2, space="PSUM")

        h_live = {}     # (chunk, li) -> produced tile (li 0 == x input)

        STEP = 8        # chunks per pipeline step

        def groups_of(rows):
            """Greedy partition of a STEP-chunk batch into dense packing
            groups of at most 128//rows chunks."""
            if rows == 0:
                return []
            cap = max(1, 128 // rows)
            out, left = [], STEP
            while left > 0:
                g = min(cap, left)
                out.append(g)
                left -= g
            return out

        def emit_load_step(t):
            for half in range(2):
                c0 = STEP * t + 4 * half
                x_t = xpool.tile([G * N_IN, 4 * chunk], F16, tag="x")
                nc.gpsimd.dma_start(
                    out=x_t[:], in_=xT[:, c0 * chunk:(c0 + 4) * chunk])
                for j in range(4):
                    h_live[(c0 + j, 0)] = x_t[:, j * chunk:(j + 1) * chunk]

        def emit_layer_step(t, li):
            a = 4 * nsin[li]                    # sin rows [0:a)
            b = 4 * (nsin[li] + ngauss[li])     # gauss rows [a:b)
            ng4 = b - a
            kdim = G * N_IN if li == 0 else 128
            cb = 8 * li
            chunks = list(range(STEP * t, STEP * (t + 1)))
            sin_gs = groups_of(a)
            gau_gs = groups_of(ng4)
            # chunk index -> (group id, slot k, group size) per class
            def slots(gs):
                mp = {}
                i = 0
                for gi, gsz in enumerate(gs):
                    for k in range(gsz):
                        mp[i] = (gi, k, gsz)
                        i += 1
                return mp
            sin_slot = slots(sin_gs)
            gau_slot = slots(gau_gs)
            sg_tiles, gp_tiles = {}, {}
            hs = {}

            def flush_sin(gi, members):
                sg = sg_tiles.pop(gi)
                um_t = spool.tile([128, chunk], F32, tag="um")
                nc.vector.tensor_scalar(
                    um_t[:], sg[:],
                    col_t[:, cb + 2:cb + 3], INV_2PI,
                    mybir.AluOpType.add, mybir.AluOpType.mult)
                k_t = spool.tile([128, chunk], F32, tag="k")
                nc.vector.tensor_scalar(
                    k_t[:], um_t[:], MAGIC, -MAGIC,
                    mybir.AluOpType.add, mybir.AluOpType.add)
                m_t = spool.tile([128, chunk], F32, tag="m")
                nc.vector.scalar_tensor_tensor(
                    m_t[:], k_t[:], -TWO_PI, sg[:],
                    mybir.AluOpType.mult, mybir.AluOpType.add)
                sr = srpool.tile([128, chunk], F16, tag="sr")
                nc.scalar.activation(
                    sr[:], m_t[:],
                    mybir.ActivationFunctionType.Sin,
                    bias=col_t[:, cb + 3:cb + 4],
                )
                for k, c in enumerate(members):
                    nc.sync.dma_start(
                        out=hs[c][0:a, :], in_=sr[a * k:a * k + a, :])

            def flush_gauss(gi, members):
                gp = gp_tiles.pop(gi)
                nc.scalar.activation(
                    gp[:], gp[:],
                    mybir.ActivationFunctionType.Square,
                    bias=col_t[:, cb + 5:cb + 6],
                    scale=col_t[:, cb + 4:cb + 5],
                )
                gr = grpool.tile([128, chunk], F16, tag="gr")
                nc.scalar.activation(
                    gr[:], gp[:],
                    mybir.ActivationFunctionType.Tanh,
                    bias=col_t[:, 26:27],
                    scale=col_t[:, 25:26],
                )
                for k, c in enumerate(members):
                    nc.gpsimd.dma_start(
                        out=hs[c][a:b, :], in_=gr[ng4 * k:ng4 * k + ng4, :])

            # ---- per chunk: matmul, stage, tanh, gathers; flush groups
            # as soon as their last member is staged ----
            for i, c in enumerate(chunks):
                h_prev = h_live.pop((c, li))
                ps = ppool.tile([128, chunk], F32, tag="pre")
                for hh in range(chunk // 512):
                    sl = slice(hh * 512, (hh + 1) * 512)
                    nc.tensor.matmul(
                        ps[:, sl],
                        w_tiles[li][0:kdim, :],
                        h_prev[0:kdim, sl],
                        start=True, stop=True,
                    )
                if b:
                    # PSUM is not DMA-addressable: stage to SBUF once, then
                    # dense-gather via DMA.
                    zb = zbpool.tile([128, chunk], F32, tag="zb")
                    nc.vector.tensor_copy(zb[:], ps[:])
                h = hpool.tile([128, chunk], F16, tag="h")
                hs[c] = h
                nc.scalar.activation(
                    h[:], ps[:], mybir.ActivationFunctionType.Tanh,
                    bias=col_t[:, cb + 1:cb + 2],
                    scale=col_t[:, cb + 0:cb + 1],
                )
                h_live[(c, li + 1)] = h
                if a:
                    gi, k, gsz = sin_slot[i]
                    if k == 0:
                        sg_tiles[gi] = sgpool.tile([128, chunk], F32,
                                                   tag="sg", name="sg")
                    w = a if k < gsz - 1 else min(128 - a * k, 128)
                    nc.gpsimd.dma_start(
                        out=sg_tiles[gi][a * k:a * k + w, :],
                        in_=zb[0:w, :])
                    if k == gsz - 1:
                        flush_sin(gi, chunks[i - gsz + 1:i + 1])
                if ng4:
                    gi, k, gsz = gau_slot[i]
                    if k == 0:
                        gp_tiles[gi] = gppool.tile([128, chunk], F32,
                                                   tag="gp", name="gp")
                    w = ng4 if k < gsz - 1 else min(128 - ng4 * k, 128 - a)
                    nc.sync.dma_start(
                        out=gp_tiles[gi][ng4 * k:ng4 * k + w, :],
                        in_=zb[a:a + w, :])
                    if k == gsz - 1:
                        flush_gauss(gi, chunks[i - gsz + 1:i + 1])

        def emit_out_step(t):
            mm_n = chunk // 2
            for p in range(STEP * t // 2, STEP * (t + 1) // 2):
                pso = oppool.tile([128, mm_n], F32, tag="preo", name="pso")
                for idx, c in enumerate((2 * p, 2 * p + 1)):
                    h_prev = h_live.pop((c, 3))
                    for hh in range(2):
                        q = 2 * idx + hh
                        nc.tensor.matmul(
                            pso[32 * q:32 * q + 32, :],
                            wo_t,
                            h_prev[:, hh * mm_n:(hh + 1) * mm_n],
                            start=True, stop=True,
                            tile_position=(0, 32 * q),
                        )
                osb = opool.tile([128, mm_n], F32, tag="osb")
                nc.scalar.activation(
                    osb[:], pso[:],
                    mybir.ActivationFunctionType.Tanh,
                    bias=col_t[:, 24:25],
                )
                nc.sync.dma_start(
                    out=yT[:, p * mm_n:(p + 1) * mm_n], in_=osb[:])

        nstep = nchunk // STEP
        for t in range(nstep + 3):
            if t < nstep:
                emit_load_step(t)
                emit_layer_step(t, 0)
            if 1 <= t and t - 1 < nstep:
                emit_layer_step(t - 1, 1)
            if 2 <= t and t - 2 < nstep:
                emit_layer_step(t - 2, 2)
            if 3 <= t and t - 3 < nstep:
                emit_out_step(t - 3)

        for p in (oppool, ppool, opool, grpool, gppool, srpool, sgpool,
                  zbpool, spool, hpool, xpool, cpool):
            p.release()

    nc.compile()
    return nc


_PROGRAM_CACHE = {}


def _get_program(nsin, ngauss, p_core=P_CORE, chunk=CHUNK):
    key = (tuple(nsin), tuple(ngauss), p_core, chunk)
    if key not in _PROGRAM_CACHE:
        _PROGRAM_CACHE[key] = _build_program(nsin, ngauss, p_core, chunk)
    return _PROGRAM_CACHE[key]


def make_in_maps(inputs, plan, p_core=P_CORE, n_cores=N_CORES):
    """Shard + transpose the pixel data; replicate constants."""
    x = np.asarray(inputs["inputs"], dtype=np.float32).astype(np.float16)
    pg = p_core // G
    wst = np.zeros((128, 416), dtype=np.float16)
    wst[0:G * N_IN, 0:128] = plan.lhsT[0]
    wst[:, 128:256] = plan.lhsT[1]
    wst[:, 256:384] = plan.lhsT[2]
    wst[:, 384:416] = plan.lhsT_out
    cst = plan.colblk
    in_maps = []
    for core in range(n_cores):
        xc = x[core * p_core:(core + 1) * p_core]          # [p_core, 12]
        xg = xc.reshape(G, pg, N_IN)                        # [G, pg, 12]
        xT = np.ascontiguousarray(
            xg.transpose(0, 2, 1).reshape(G * N_IN, pg))    # [48, pg]
        in_maps.append({"xT": xT, "wst": wst, "cst": cst})
    return in_maps


def assemble_output(results, p_core=P_CORE, n_cores=N_CORES):
    pg = p_core // G
    npair = pg // CHUNK // 2
    mm_n = CHUNK // 2
    out = np.empty((p_core * n_cores, N_OUT), dtype=np.float32)
    for core in range(n_cores):
        yT = results[core]["yT"]                  # [128, pg//4]
        # row 32*(2*idx+hh) + 3*g + o, col p*512+j ->
        #   pixel (g, (2p+idx)*CHUNK + hh*mm_n + j), output o
        y = yT.reshape(4, 32, npair, mm_n)        # [q, 3g+o, p, j]
        y = y.reshape(4, 32, npair * mm_n)[:, :12]
        y = y.reshape(4, 4, 3, npair, mm_n)       # [q, g, o, p, j]
        q = np.arange(4)
        oc = np.empty((G, npair, 2, 2, mm_n, N_OUT), dtype=np.float32)
        for qq in range(4):
            idx, hh = qq // 2, qq % 2
            oc[:, :, idx, hh, :, :] = y[qq].transpose(0, 2, 3, 1)
        out[core * p_core:(core + 1) * p_core] = oc.reshape(p_core, N_OUT)
    return out


def make_plan(inputs):
    return _Plan(
        inputs["bias_in"], inputs["W1"], inputs["b1"], inputs["act1"],
        inputs["W2"], inputs["b2"], inputs["act2"],
        inputs["W3"], inputs["b3"], inputs["act3"],
        inputs["Wout"], inputs["bout"])


def run(inputs, trace=False, **spmd_kwargs):
    plan = make_plan(inputs)
    nc = _get_program(plan.nsin, plan.ngauss)
    in_maps = make_in_maps(inputs, plan)
    res = run_bass_kernel_spmd(nc, in_maps, list(range(N_CORES)),
                               trace=trace, **spmd_kwargs)
    return assemble_output(res.results), res


def kernel(**inputs) -> np.ndarray:
    out, _ = run(inputs, trace=False)
    return out
